# revision 7
# baseline (speedup 1.0000x reference)
"""Trainium2 Bass kernel for ContractExpand (segment_reduce).

For each scale r in (1,2,4,10,25): segment-sum groups of r consecutive rows,
relu(Linear_r)/r, broadcast back to rows, concat all scales along rows.

Strategy: pure data parallel over 8 NeuronCores (row-sharded, 12500 rows per
core). The host passes x TRANSPOSED ([301, 12500] fp16, row 300 = ones), so
the kernel's input loads are large contiguous DMAs and the contraction dim
is already on partitions. Output is written fp16 (the host upcasts), halving
write traffic vs fp32.

Per core, per superchunk (graded sizes for pipeline ramp/tail):
  load:   xT k-slices [128|128|45, sc] from DRAM on the SP HWDGE ring.
  sums:   DVE fp16 segment sums. r2 = x[even]+x[odd], r4 = r2[e]+r2[o],
          r10 = r5[e]+r5[o] via scalar_tensor_tensor (2 reads/cycle);
          r5 = reduce(x,5), r25 = reduce(r5,5). Ones row reduces to r,
          compensated by the host-side bias row b/r^2 folded into the GEMM.
  mm:     psum accumulates 3 k-slice matmuls (K=128+128+45 incl bias).
          For r=1, groups are processed in supertiles of 512 with stride-4
          column interleave so each SBUF partition ends up holding 4
          consecutive output rows -> 2400B store descriptors instead of
          600B (per-packet DMA overhead is the write bottleneck).
  act:    scalar relu psum -> fp16 h tile; two psum tiles (one [128,1024]
          2-bank allocation) are drained by ONE activation instruction to
          amortize the ~200ns psum-read latency per instruction.
  rep:    DVE replicates the relu result rep_r times along the free dim
          so store descriptors reach 1200-3000B.
  store:  consolidated dmas issued from gpsimd (SWDGE: ~0.34ns/descriptor
          vs ~9ns on the HWDGE sequencer path), on a separate queue from
          the loads so prefetches are never stuck behind store bursts.
"""

import sys

import numpy as np

if "/opt/trn_rl_repo" not in sys.path:
    sys.path.insert(0, "/opt/trn_rl_repo")

from contextlib import ExitStack

import concourse.tile as tile
from concourse import bacc, mybir

DIM = 300
KEXT = 301  # 300 dims + ones row
SCALES = (1, 2, 4, 10, 25)
REP = {1: 1, 2: 2, 4: 4, 10: 5, 25: 5}  # SBUF replication per scale
N_TOTAL = 100000
N_CORES = 8
R_CORE = N_TOTAL // N_CORES  # 12500
SCS = [1000, 2000, 2500, 2500, 2500, 2000]  # graded superchunks
SC_MAX = max(SCS)
KSLICES = [(0, 128), (128, 256), (256, 301)]  # sizes 128, 128, 45
F32 = mybir.dt.float32
DT = mybir.dt.float16
NPDT = np.float16
AXX = mybir.AxisListType.X
ALU = mybir.AluOpType


def _pad(x):
    return (x + 63) & ~63


# tmpT free-dim regions sized for the largest superchunk
B2 = 0
B4 = B2 + _pad(SC_MAX // 2)
B5 = B4 + _pad(SC_MAX // 4)
B10 = B5 + _pad(SC_MAX // 5)
B25 = B10 + _pad(SC_MAX // 10)
MAXR = B25 + _pad(SC_MAX // 25)
BASE = {2: B2, 4: B4, 10: B10, 25: B25}

# h tile widths (elems/partition) for the largest superchunk
NBLK = {r: (SC_MAX // r + 127) // 128 for r in SCALES}


def _emit(ctx, tc, xt_ap, wt_ap, out_ap, rows):
    nc = tc.nc
    assert sum(SCS) == rows

    singles = ctx.enter_context(tc.tile_pool(name="singles", bufs=1))
    # [128,1024] fp32 = 2 psum banks; two matmul targets per allocation
    ppool = ctx.enter_context(tc.tile_pool(name="pp", bufs=4, space="PSUM"))

    wt_sb = []
    for i in range(len(SCALES)):
        per_s = []
        for s, (k0, k1) in enumerate(KSLICES):
            t = singles.tile([k1 - k0, DIM], DT, tag=f"wt{i}_{s}")
            nc.sync.dma_start(out=t[:], in_=wt_ap[i, k0:k1, :])
            per_s.append(t)
        wt_sb.append(per_s)

    xT = [
        [
            singles.tile([k1 - k0, SC_MAX], DT, tag=f"xT{b}_{s}", name=f"xT{b}_{s}")
            for s, (k0, k1) in enumerate(KSLICES)
        ]
        for b in range(2)
    ]
    tmpT = [
        [
            singles.tile([k1 - k0, MAXR], DT, tag=f"tm{b}_{s}", name=f"tm{b}_{s}")
            for s, (k0, k1) in enumerate(KSLICES)
        ]
        for b in range(2)
    ]
    hT = [
        [
            singles.tile(
                [128, NBLK[r] * REP[r] * DIM], DT, tag=f"h{b}_{si}", name=f"h{b}_{si}"
            )
            for si, r in enumerate(SCALES)
        ]
        for b in range(2)
    ]

    ks = [(s, k1 - k0) for s, (k0, k1) in enumerate(KSLICES)]

    def load_sc(sci, row0):
        xb = xT[sci % 2]
        sc = SCS[sci]
        for s, (k0, k1) in enumerate(KSLICES):
            nc.sync.dma_start(
                out=xb[s][: k1 - k0, :sc],
                in_=xt_ap[k0:k1, row0 : row0 + sc],
            )

    row0 = 0
    next_row0 = 0
    for sci, sc in enumerate(SCS):
        xb = xT[sci % 2]
        tb = tmpT[sci % 2]
        hb = hT[sci % 2]
        if sci == 0:
            load_sc(0, 0)
            next_row0 = sc
        if sci + 1 < len(SCS):
            load_sc(sci + 1, next_row0)
            next_row0 += SCS[sci + 1]

        w2, w4, w5, w10, w25 = (sc // r for r in (2, 4, 5, 10, 25))

        # --- DVE segment sums, op-major so PE deps become ready early ---
        def _pairsum(tm, ksz, dst0, w_dst, srcv):
            nc.vector.scalar_tensor_tensor(
                out=tm[:ksz, dst0 : dst0 + w_dst].unsqueeze(2),
                in0=srcv[:, :, 0:1],
                scalar=0.0,
                in1=srcv[:, :, 1:2],
                op0=ALU.bypass,
                op1=ALU.add,
            )

        lp = nc.allow_low_precision(reason="fp16 segment sums feed fp16 matmul")
        lp.__enter__()
        for s, ksz in ks:
            src = xb[s][:ksz, :sc]
            _pairsum(
                tb[s], ksz, B2, w2, src.rearrange("p (g two) -> p g two", two=2)
            )
        for s, ksz in ks:
            r2v = tb[s][:ksz, B2 : B2 + w2].rearrange(
                "p (g two) -> p g two", two=2
            )
            _pairsum(tb[s], ksz, B4, w4, r2v)
        for s, ksz in ks:
            nc.vector.reduce_sum(
                out=tb[s][:ksz, B5 : B5 + w5],
                in_=xb[s][:ksz, :sc].rearrange("p (g r) -> p g r", r=5),
                axis=AXX,
            )
        for s, ksz in ks:
            r5v = tb[s][:ksz, B5 : B5 + w5].rearrange(
                "p (g two) -> p g two", two=2
            )
            _pairsum(tb[s], ksz, B10, w10, r5v)
        for s, ksz in ks:
            nc.vector.reduce_sum(
                out=tb[s][:ksz, B25 : B25 + w25],
                in_=tb[s][:ksz, B5 : B5 + w5].rearrange("p (g r) -> p g r", r=5),
                axis=AXX,
            )
        lp.__exit__(None, None, None)

        # --- matmul units per scale ---
        # A unit = (lhsT per kslice, M, h column base). Units are drained in
        # psum-pairs: 2 units -> one [128,1024] psum alloc -> one activation.
        for si, r in enumerate(SCALES):
            width = sc // r
            rep = REP[r]
            bw = rep * DIM
            h = hb[si]
            units = []  # (lhsT_fn(s, ksz), M, cb)

            if r == 1:
                nst = width // 512
                for t in range(nst):
                    base = t * 512
                    for e in range(4):

                        def lf(s, ksz, base=base, e=e):
                            v = xb[s][:ksz, base : base + 512].rearrange(
                                "p (g four) -> p four g", four=4
                            )
                            return v[:, e : e + 1, :].rearrange(
                                "p one g -> p (one g)"
                            )

                        units.append((lf, 128, t * 1200 + e * 300))
                t0 = nst * 512
                cb0 = nst * 1200
                for mi, c0 in enumerate(range(t0, width, 128)):
                    M = min(128, width - c0)

                    def lf(s, ksz, c0=c0, M=M):
                        return xb[s][:ksz, c0 : c0 + M]

                    units.append((lf, M, cb0 + mi * 300))
            else:
                b0 = BASE[r]
                for mi, c0 in enumerate(range(0, width, 128)):
                    M = min(128, width - c0)

                    def lf(s, ksz, b0=b0, c0=c0, M=M):
                        return tb[s][:ksz, b0 + c0 : b0 + c0 + M]

                    units.append((lf, M, mi * bw))

            # drain units in pairs (equal M) with one activation each
            ui = 0
            while ui < len(units):
                pair = None
                if ui + 1 < len(units) and units[ui][1] == units[ui + 1][1]:
                    pair = (units[ui], units[ui + 1])
                pp = ppool.tile([128, 1024], F32, tag="pp")
                if pair:
                    (lfA, M, cbA), (lfB, _, cbB) = pair
                    for s, ksz in ks:
                        nc.tensor.matmul(
                            pp[:M, 0:DIM], lfA(s, ksz), wt_sb[si][s][:],
                            start=(s == 0), stop=(s == 2),
                        )
                    for s, ksz in ks:
                        nc.tensor.matmul(
                            pp[:M, 512 : 512 + DIM], lfB(s, ksz), wt_sb[si][s][:],
                            start=(s == 0), stop=(s == 2),
                        )
                    stride = cbB - cbA
                    pin = pp[:M, 0:1024].rearrange(
                        "p (two half) -> p two half", half=512
                    )[:, :, 0:DIM]
                    pout = h[:M, cbA : cbA + 2 * stride].rearrange(
                        "p (two blk) -> p two blk", blk=stride
                    )[:, :, 0:DIM]
                    nc.scalar.activation(
                        out=pout, in_=pin,
                        func=mybir.ActivationFunctionType.Relu,
                    )
                    # replicate both units with strided copies
                    done = 1
                    while done < rep:
                        cnt = min(done, rep - done)
                        hv = h[:M, cbA : cbA + 2 * stride].rearrange(
                            "p (two blk) -> p two blk", blk=stride
                        )
                        nc.vector.tensor_copy(
                            out=hv[:, :, done * DIM : (done + cnt) * DIM],
                            in_=hv[:, :, 0 : cnt * DIM],
                        )
                        done += cnt
                    ui += 2
                else:
                    lfA, M, cbA = units[ui]
                    for s, ksz in ks:
                        nc.tensor.matmul(
                            pp[:M, 0:DIM], lfA(s, ksz), wt_sb[si][s][:],
                            start=(s == 0), stop=(s == 2),
                        )
                    nc.scalar.activation(
                        out=h[:M, cbA : cbA + DIM], in_=pp[:M, 0:DIM],
                        func=mybir.ActivationFunctionType.Relu,
                    )
                    done = 1
                    while done < rep:
                        cnt = min(done, rep - done)
                        nc.vector.tensor_copy(
                            out=h[:M, cbA + done * DIM : cbA + (done + cnt) * DIM],
                            in_=h[:M, cbA : cbA + cnt * DIM],
                        )
                        done += cnt
                    ui += 1

            # --- stores (SWDGE via gpsimd) ---
            jr = r // rep
            orow0 = si * rows + row0
            if r == 1:
                nst = width // 512
                if nst > 0:
                    dst = out_ap[orow0 : orow0 + nst * 512, :].rearrange(
                        "(m p e) d -> p m (e d)", p=128, e=4
                    )
                    src = h[:128, : nst * 1200].rearrange(
                        "p (m ed) -> p m ed", ed=1200
                    )
                    nc.gpsimd.dma_start(out=dst, in_=src)
                t0 = nst * 512
                tail = width - t0
                F = tail // 128
                Mt = tail - F * 128
                cb0 = nst * 1200
                if F > 0:
                    dst = out_ap[
                        orow0 + t0 : orow0 + t0 + F * 128, :
                    ].rearrange("(m p) d -> p m d", p=128)
                    src = h[:128, cb0 : cb0 + F * 300].rearrange(
                        "p (m d) -> p m d", d=300
                    )
                    nc.gpsimd.dma_start(out=dst, in_=src)
                if Mt > 0:
                    dst = out_ap[orow0 + t0 + F * 128 : orow0 + width, :]
                    nc.gpsimd.dma_start(
                        out=dst,
                        in_=h[:Mt, cb0 + F * 300 : cb0 + (F + 1) * 300],
                    )
            elif jr == 1:
                F = width // 128
                Mt = width - F * 128
                if F > 0:
                    dst = out_ap[orow0 : orow0 + F * 128 * r, :].rearrange(
                        "(m p e) d -> p m (e d)", p=128, e=rep
                    )
                    src = h[:128, : F * bw].rearrange("p (m ed) -> p m ed", ed=bw)
                    nc.gpsimd.dma_start(out=dst, in_=src)
                if Mt > 0:
                    orowt = orow0 + F * 128 * r
                    dst = out_ap[orowt : orowt + Mt * r, :].rearrange(
                        "(p e) d -> p (e d)", e=rep
                    )
                    nc.gpsimd.dma_start(
                        out=dst, in_=h[:Mt, F * bw : F * bw + bw]
                    )
            else:
                for mi, c0 in enumerate(range(0, width, 128)):
                    M = min(128, width - c0)
                    cb = mi * bw
                    or_m = orow0 + c0 * r
                    dst = out_ap[or_m : or_m + M * r, :].rearrange(
                        "(p j e) d -> p j (e d)", j=jr, e=rep
                    )
                    src = (
                        h[:M, cb : cb + bw]
                        .unsqueeze(1)
                        .broadcast_to([M, jr, bw])
                    )
                    nc.gpsimd.dma_start(out=dst, in_=src)

        row0 += sc


def build_nc(rows=R_CORE):
    nc = bacc.Bacc("TRN2", target_bir_lowering=False)
    xt = nc.declare_dram_parameter("xt", [KEXT, rows], DT, isOutput=False)
    wt = nc.declare_dram_parameter(
        "wt", [len(SCALES), KEXT, DIM], DT, isOutput=False
    )
    out = nc.declare_dram_parameter(
        "out", [len(SCALES) * rows, DIM], DT, isOutput=True
    )
    with tile.TileContext(nc) as tc:
        with ExitStack() as ctx:
            _emit(ctx, tc, xt.ap(), wt.ap(), out.ap(), rows)
    nc.compile()
    return nc


def make_wt(Ws, bs):
    """[5, 301, 300]: [W_r.T / r ; b_r / r^2] (ones row reduces to r)."""
    wt = np.empty((len(SCALES), KEXT, DIM), np.float32)
    for i, r in enumerate(SCALES):
        wt[i, :DIM, :] = np.asarray(Ws[i], np.float32).T / r
        wt[i, DIM, :] = np.asarray(bs[i], np.float32) / (r * r)
    return wt


def make_xt(x_shard):
    """[n, 300] fp32 -> [301, n] fp16 with ones in row 300."""
    n = len(x_shard)
    xt = np.empty((KEXT, n), NPDT)
    xt[:DIM, :] = x_shard.astype(NPDT).T
    xt[DIM, :] = 1.0
    return np.ascontiguousarray(xt)


_NC_CACHE = {}


def _get_nc(rows):
    if rows not in _NC_CACHE:
        _NC_CACHE[rows] = build_nc(rows)
    return _NC_CACHE[rows]


def run_cores(inputs_c_e, Ws, bs, trace=False, **kw):
    """Shard, run on the 8 NeuronCores, gather. Returns (full_out, results)."""
    from concourse.bass_utils import run_bass_kernel_spmd

    x = np.ascontiguousarray(np.asarray(inputs_c_e, np.float32))
    n = x.shape[0]
    assert n == N_TOTAL
    wt = make_wt(Ws, bs).astype(NPDT)
    nc = _get_nc(R_CORE)
    in_maps = [
        {"xt": make_xt(x[c * R_CORE : (c + 1) * R_CORE]), "wt": wt}
        for c in range(N_CORES)
    ]
    res = run_bass_kernel_spmd(nc, in_maps, list(range(N_CORES)), trace=trace, **kw)
    full = np.empty((len(SCALES) * n, DIM), np.float32)
    for si in range(len(SCALES)):
        for c in range(N_CORES):
            full[si * n + c * R_CORE : si * n + (c + 1) * R_CORE] = res.results[c][
                "out"
            ][si * R_CORE : (si + 1) * R_CORE]
    return full, res


def kernel(inputs_c_e, Ws, bs):
    full, _ = run_cores(inputs_c_e, Ws, bs)
    return full
